# revision 1
# baseline (speedup 1.0000x reference)
"""ArcFace loss on 8 TRN2 NeuronCores (vocab/tensor-parallel over classes).

Math (per reference):
    cos = normalize(emb) @ normalize(W).T            [B, C]
    phi applied at the label column only (ArcFace margin)
    loss = mean CE(64 * modified cos, labels)

The margin only matters at the label position, so the full phi/sine
[B, C] matrices are never materialized.  Each core owns a contiguous
class shard (host staging: transpose to [D, C/8], bf16 cast, zero-pad
to a multiple of 512 classes).  On device each core normalizes its
class columns (sum-of-squares via an all-ones matmul, rsqrt as
exp(-0.5*ln(x)) so one ACT table set covers the whole kernel), runs the
bf16 matmul against the normalized/transposed embeddings, and folds
exp(64*cos - 16) + per-row sum into single Activation instructions
(constant-bias flash softmax: valid because max |logit| << 48 here, and
the zero-padded classes contribute exp(-16) each, ~4e-4 relative).
The label-column correction and the t = 64*phi(label) term are computed
redundantly on every core in f32 from a host-gathered weight[labels]
(the correction is scaled by 1/8 so the cross-core sum applies it
once).  One tiny AllGather (4 KB) exchanges the per-row partial sums;
each core then reduces, takes ln, and emits the scalar loss.
"""

import math
import numpy as np
import ml_dtypes

import concourse.bass as bass
import concourse.mybir as mybir
from concourse import bacc, bass_isa, tile, masks
from concourse.bass_utils import run_bass_kernel_spmd

# Pin every ACT instruction to the one table set that covers all functions
# this kernel uses (exp, ln, square, identity, copy) so the activation
# table is loaded once instead of thrashing between per-function sets.
_ACT_SET = "natural_log_exp_and_others"
try:
    _orig_get_act_tables = bacc.get_activation_tables

    def _pinned_act_tables(arch):
        tables = _orig_get_act_tables(arch)
        if _ACT_SET in tables:
            return {name: (fns if name == _ACT_SET else set())
                    for name, fns in tables.items()}
        return tables

    bacc.get_activation_tables = _pinned_act_tables
except AttributeError:
    pass

N_CORES = 8
B = 1024
D = 512
C = 100000
C_PER = C // N_CORES          # 12500
CP = 12800                    # per-core classes padded to 25 * 512
CB = 512                      # matmul free-dim block (one PSUM bank)
SUPER_CB = 3                  # class blocks per exp super-block (3 banks)
SCALE = 64.0
MARGIN = 0.5
EXP_BIAS = -16.0
EPS = 1e-12

FP32 = mybir.dt.float32
BF16 = mybir.dt.bfloat16
AF = mybir.ActivationFunctionType
ALU = mybir.AluOpType
X = mybir.AxisListType.X

COS_M = math.cos(MARGIN)
SIN_M = math.sin(MARGIN)
TH = math.cos(math.pi - MARGIN)
MM = math.sin(math.pi - MARGIN) * MARGIN


def _supers(n_blocks: int, super_cb: int):
    """[(first_block, n_cb), ...] covering n_blocks class blocks."""
    out = []
    b = 0
    while b < n_blocks:
        n = min(super_cb, n_blocks - b)
        out.append((b, n))
        b += n
    return out


def build_graph(b=B, cp=CP, super_cb=SUPER_CB):
    m_tiles = b // 128
    k_chunks = D // 128
    n_blocks = cp // CB
    supers = _supers(n_blocks, super_cb)
    n_sup = len(supers)

    nc = bacc.Bacc("TRN2", target_bir_lowering=False, debug=False,
                   num_devices=N_CORES)
    emb = nc.dram_tensor("emb", [b, D], FP32, kind="ExternalInput")
    wt = nc.dram_tensor("wt", [D, cp], BF16, kind="ExternalInput")
    wlab = nc.dram_tensor("wlab", [b, D], FP32, kind="ExternalInput")
    out = nc.dram_tensor("out", [1, 1], FP32, kind="ExternalOutput")

    emb_ap = emb.ap()
    wt_ap = wt.ap()
    wlab_ap = wlab.ap()

    with tile.TileContext(nc) as tc:
        with (
            tc.tile_pool(name="const", bufs=1) as cpool,
            tc.tile_pool(name="persist", bufs=1) as pp,
            tc.tile_pool(name="small", bufs=4) as sp,
            tc.tile_pool(name="scr", bufs=4) as scr,
        ):
            ones_b = cpool.tile([128, 128], BF16, tag="ones_b")
            nc.vector.memset(ones_b[:], 1.0)
            ident = cpool.tile([128, 128], BF16, tag="ident")
            ones_f = cpool.tile([128, 1], FP32, tag="ones_f")
            bias_n = cpool.tile([128, 1], FP32, tag="bias_n")

            # persistent state
            ehat = [pp.tile([128, D], FP32, tag=f"ehat{m}", name=f"ehat{m}")
                    for m in range(m_tiles)]
            ehT = [pp.tile([128, b], BF16, tag=f"ehT{k}", name=f"ehT{k}")
                   for k in range(k_chunks)]
            acc = [pp.tile([128, n_sup], FP32, tag=f"acc{m}", name=f"acc{m}")
                   for m in range(m_tiles)]
            cosl = pp.tile([128, m_tiles], FP32, tag="cosl")
            tlab = pp.tile([128, m_tiles], FP32, tag="tlab")
            delta = pp.tile([128, m_tiles], FP32, tag="delta")
            sloc = pp.tile([128, m_tiles], FP32, tag="sloc")

            # ---- phase 1: streamed classes: norm, matmul, exp-accum ----
            with (
                tc.tile_pool(name="psum_s", bufs=2, space="PSUM") as pss,
                tc.tile_pool(name="wpool", bufs=2 * k_chunks) as wp,
                tc.tile_pool(name="whpool", bufs=1) as whp,
                tc.tile_pool(name="sqpool", bufs=6) as sqp,
                tc.tile_pool(name="rwpool", bufs=2) as rwp,
                tc.tile_pool(name="expool", bufs=2) as exp_p,
            ):
                # triple-buffered normalized-transposed weight tiles
                NBUF = 3
                wh = {}
                for j in range(NBUF):
                    for cb in range(super_cb):
                        for k in range(k_chunks):
                            wh[(j, cb, k)] = whp.tile([128, CB], BF16, tag=f"wh{j}_{cb}_{k}",
                                                      name=f"wh{j}_{cb}_{k}")

                def prep(si):
                    cb0, ncb = supers[si]
                    j = si % NBUF
                    wtbig = {}
                    ssq_sb = rwp.tile([128, super_cb * CB], FP32, tag="ssq_sb",
                                      name=f"ssq_sb{si}")
                    for cb in range(ncb):
                        co = (cb0 + cb) * CB
                        wtb = wp.tile([128, k_chunks, CB], BF16, tag="wt",
                                      name=f"wt{si}_{cb}")
                        nc.sync.dma_start(
                            wtb[:], wt_ap[:, co:co + CB].rearrange(
                                "(k p) x -> p k x", p=128))
                        wtbig[cb] = wtb
                        ps = pss.tile([128, CB], FP32, tag="ps",
                                      name=f"ps{si}_{cb}")
                        for k in range(k_chunks):
                            sqk = sqp.tile([128, CB], BF16, tag="sq",
                                           name=f"sq{si}_{cb}_{k}")
                            nc.vector.tensor_tensor(sqk[:], wtb[:, k, :],
                                                    wtb[:, k, :], ALU.mult)
                            nc.tensor.matmul(ps[:], ones_b[:], sqk[:],
                                             start=(k == 0),
                                             stop=(k == k_chunks - 1))
                        nc.vector.tensor_scalar_max(
                            ssq_sb[:, cb * CB:(cb + 1) * CB], ps[:], 1e-24)
                    lnq_s = rwp.tile([128, super_cb * CB], FP32, tag="lnq_s",
                                     name=f"lnq_s{si}")
                    nc.scalar.activation(lnq_s[:, :ncb * CB],
                                         ssq_sb[:, :ncb * CB], AF.Ln)
                    rw_s = rwp.tile([128, super_cb * CB], BF16, tag="rw_s",
                                    name=f"rw_s{si}")
                    nc.scalar.activation(rw_s[:, :ncb * CB],
                                         lnq_s[:, :ncb * CB], AF.Exp,
                                         scale=-0.5)
                    for cb in range(ncb):
                        for k in range(k_chunks):
                            nc.vector.tensor_tensor(
                                wh[(j, cb, k)][:], wtbig[cb][:, k, :],
                                rw_s[:, cb * CB:(cb + 1) * CB], ALU.mult)

                for si in range(min(2, n_sup)):
                    prep(si)

                # deferred constants (keeps the DVE free for the first
                # weight-square tiles so the PE starts earlier)
                masks.make_identity(nc, ident[:])
                nc.vector.memset(ones_f[:], 1.0 / b)
                nc.vector.memset(bias_n[:], EXP_BIAS)

                # ---- phase 0: embedding normalization + PE transpose ----
                with tc.tile_pool(name="psum_t", bufs=4, space="PSUM") as pst:
                    for m in range(m_tiles):
                        et = scr.tile([128, D], FP32, tag="et")
                        nc.sync.dma_start(et[:], emb_ap[m * 128:(m + 1) * 128, :])
                        sq_s = scr.tile([128, D], FP32, tag="sq_s")
                        ssq = sp.tile([128, 1], FP32, tag="ssq")
                        nc.scalar.activation(sq_s[:], et[:], AF.Square,
                                             accum_out=ssq[:])
                        ssqc = sp.tile([128, 1], FP32, tag="ssqc")
                        nc.vector.tensor_scalar_max(ssqc[:], ssq[:], EPS * EPS)
                        lnq = sp.tile([128, 1], FP32, tag="lnq")
                        nc.scalar.activation(lnq[:], ssqc[:], AF.Ln)
                        re = sp.tile([128, 1], FP32, tag="re")
                        nc.scalar.activation(re[:], lnq[:], AF.Exp, scale=-0.5)
                        nc.vector.tensor_scalar_mul(ehat[m][:], et[:], re[:])
                        eb = scr.tile([128, D], BF16, tag="eb")
                        nc.vector.tensor_copy(eb[:], ehat[m][:])
                        for k in range(k_chunks):
                            pt = pst.tile([128, 128], BF16, tag="pt")
                            nc.tensor.transpose(
                                pt[:], eb[:, k * 128:(k + 1) * 128], ident[:])
                            nc.vector.tensor_copy(
                                ehT[k][:, m * 128:(m + 1) * 128], pt[:])

                # ---- phase 2: label column correction (f32, replicated) ----
                for m in range(m_tiles):
                    wlt = scr.tile([128, D], FP32, tag="wlt")
                    nc.sync.dma_start(wlt[:], wlab_ap[m * 128:(m + 1) * 128, :])
                    sq_s = scr.tile([128, D], FP32, tag="sq_s")
                    ssql = sp.tile([128, 1], FP32, tag="ssql")
                    nc.scalar.activation(sq_s[:], wlt[:], AF.Square,
                                         accum_out=ssql[:])
                    ssqlc = sp.tile([128, 1], FP32, tag="ssqlc")
                    nc.vector.tensor_scalar_max(ssqlc[:], ssql[:], EPS * EPS)
                    lnl = sp.tile([128, 1], FP32, tag="lnl")
                    nc.scalar.activation(lnl[:], ssqlc[:], AF.Ln)
                    rl = sp.tile([128, 1], FP32, tag="rl")
                    nc.scalar.activation(rl[:], lnl[:], AF.Exp, scale=-0.5)
                    dsc = scr.tile([128, D], FP32, tag="dsc")
                    dotm = sp.tile([128, 1], FP32, tag="dotm")
                    nc.vector.tensor_tensor(dsc[:], ehat[m][:], wlt[:], ALU.mult)
                    nc.vector.tensor_reduce(dotm[:], dsc[:], X, ALU.add)
                    nc.vector.tensor_tensor(cosl[:, m:m + 1], dotm[:], rl[:],
                                            ALU.mult)

                c2 = sp.tile([128, m_tiles], FP32, tag="c2")
                nc.scalar.activation(c2[:], cosl[:], AF.Square)
                ss = sp.tile([128, m_tiles], FP32, tag="ss")
                nc.vector.tensor_scalar(ss[:], c2[:], -1.0, 1.0, ALU.mult, ALU.add)
                nc.vector.tensor_scalar_max(ss[:], ss[:], 1e-30)
                lns = sp.tile([128, m_tiles], FP32, tag="lns")
                nc.scalar.activation(lns[:], ss[:], AF.Ln)
                sinl = sp.tile([128, m_tiles], FP32, tag="sinl")
                nc.scalar.activation(sinl[:], lns[:], AF.Exp, scale=0.5)
                pa = sp.tile([128, m_tiles], FP32, tag="pa")
                nc.vector.tensor_scalar_mul(pa[:], cosl[:], COS_M)
                pb = sp.tile([128, m_tiles], FP32, tag="pb")
                nc.vector.tensor_scalar_mul(pb[:], sinl[:], SIN_M)
                phi = sp.tile([128, m_tiles], FP32, tag="phi")
                nc.vector.tensor_tensor(phi[:], pa[:], pb[:], ALU.subtract)
                alt = sp.tile([128, m_tiles], FP32, tag="alt")
                nc.vector.tensor_scalar_sub(alt[:], cosl[:], MM)
                msk = sp.tile([128, m_tiles], FP32, tag="msk")
                nc.vector.tensor_scalar(msk[:], cosl[:], TH, None, ALU.is_gt)
                dphi = sp.tile([128, m_tiles], FP32, tag="dphi")
                nc.vector.tensor_tensor(dphi[:], phi[:], alt[:], ALU.subtract)
                mphi = sp.tile([128, m_tiles], FP32, tag="mphi")
                nc.vector.tensor_tensor(mphi[:], msk[:], dphi[:], ALU.mult)
                phis = sp.tile([128, m_tiles], FP32, tag="phis")
                nc.vector.tensor_tensor(phis[:], alt[:], mphi[:], ALU.add)
                nc.vector.tensor_scalar_mul(tlab[:], phis[:], SCALE)
                tsum0 = sp.tile([128, 1], FP32, tag="tsum0")
                nc.vector.tensor_reduce(tsum0[:], tlab[:], X, ALU.add)
                tsum = pp.tile([128, 1], FP32, tag="tsum")
                nc.vector.tensor_scalar_add(tsum[:], tsum0[:],
                                            EXP_BIAS * b / 128.0)
                ea = sp.tile([128, m_tiles], FP32, tag="ea")
                nc.scalar.activation(ea[:], phis[:], AF.Exp, bias=bias_n[:],
                                     scale=SCALE)
                eb2 = sp.tile([128, m_tiles], FP32, tag="eb2")
                nc.scalar.activation(eb2[:], cosl[:], AF.Exp, bias=bias_n[:],
                                     scale=SCALE)
                dl = sp.tile([128, m_tiles], FP32, tag="dl")
                nc.vector.tensor_tensor(dl[:], ea[:], eb2[:], ALU.subtract)
                nc.vector.tensor_scalar_mul(delta[:], dl[:], 1.0 / N_CORES)


                psg_cm = tc.tile_pool(name="psum_g", bufs=2, space="PSUM")
                psg = psg_cm.__enter__()
                slocp = pp.tile([128, m_tiles], FP32, tag="slocp")
                for si, (cb0, ncb) in enumerate(supers):
                    j = si % NBUF
                    if si + 2 < n_sup:
                        prep(si + 2)
                    if si == n_sup - 1 and n_sup >= 2:
                        for m in range(m_tiles):
                            nc.vector.tensor_reduce(
                                slocp[:, m:m + 1], acc[m][:, :n_sup - 1],
                                X, ALU.add)
                    for m in range(m_tiles):
                        pg = psg.tile([128, super_cb * CB], FP32, tag="pg")
                        for cb in range(ncb):
                            for k in range(k_chunks):
                                nc.tensor.matmul(
                                    pg[:, cb * CB:(cb + 1) * CB],
                                    ehT[k][:, m * 128:(m + 1) * 128],
                                    wh[(j, cb, k)][:],
                                    start=(k == 0), stop=(k == k_chunks - 1))
                        ex = exp_p.tile([128, super_cb * CB], BF16, tag="ex")
                        nc.scalar.activation(
                            ex[:, :ncb * CB], pg[:, :ncb * CB], AF.Exp,
                            bias=bias_n[:], scale=SCALE,
                            accum_out=acc[m][:, si:si + 1])

                psg_cm.__exit__(None, None, None)

            # ---- phase 3: reduce, all-reduce, final scalar ----
            if n_sup >= 2:
                nc.vector.tensor_tensor(sloc[:], slocp[:], delta[:], ALU.add)
                scorr = sp.tile([128, m_tiles], FP32, tag="scorr")
                for m in range(m_tiles):
                    nc.vector.tensor_tensor(
                        scorr[:, m:m + 1], sloc[:, m:m + 1],
                        acc[m][:, n_sup - 1:n_sup], ALU.add)
            else:
                for m in range(m_tiles):
                    nc.vector.tensor_reduce(sloc[:, m:m + 1], acc[m][:],
                                            X, ALU.add)
                scorr = sp.tile([128, m_tiles], FP32, tag="scorr")
                nc.vector.tensor_tensor(scorr[:], sloc[:], delta[:], ALU.add)

            with (
                tc.tile_pool(name="dram", bufs=2, space="DRAM") as dp,
                tc.tile_pool(name="psum_f", bufs=1, space="PSUM") as psf,
            ):
                in_b = dp.tile([128, m_tiles], FP32)
                out_b = dp.tile([128 * N_CORES, m_tiles], FP32)
                nc.gpsimd.dma_start(in_b[:], scorr[:])
                nc.gpsimd.collective_compute(
                    "AllGather", ALU.bypass,
                    replica_groups=[list(range(N_CORES))],
                    ins=[in_b.opt()], outs=[out_b.opt()])
                sg_all = sp.tile([128, m_tiles, N_CORES], FP32, tag="sg_all")
                nc.scalar.dma_start(
                    sg_all[:],
                    out_b.opt().rearrange("(r p) m -> p m r", p=128))
                sg = sp.tile([128, m_tiles], FP32, tag="sg")
                nc.vector.tensor_reduce(sg[:], sg_all[:], X, ALU.add)
                logs = sp.tile([128, m_tiles], FP32, tag="logs")
                lsum = sp.tile([128, 1], FP32, tag="lsum")
                nc.scalar.activation(logs[:], sg[:], AF.Ln, accum_out=lsum[:])
                lp = sp.tile([128, 1], FP32, tag="lp")
                nc.vector.tensor_scalar(lp[:], lsum[:], tsum[:], 1.0 / b,
                                        ALU.subtract, ALU.mult)
                finb = sp.tile([128, 1], FP32, tag="finb")
                nc.gpsimd.partition_all_reduce(finb[:], lp[:], 128,
                                               bass_isa.ReduceOp.add)
                nc.gpsimd.dma_start(out.ap()[:, :], finb[:1, :1])

    nc.compile()
    return nc


def make_in_maps(embeddings, weight, labels, b=B, cp=CP):
    emb = np.ascontiguousarray(np.asarray(embeddings, np.float32))
    w = np.asarray(weight, np.float32)
    lab = np.asarray(labels).astype(np.int64)
    c, d = w.shape
    c_per = c // N_CORES
    wlab = np.ascontiguousarray(w[lab])
    wT = w.T  # [D, C]
    in_maps = []
    for i in range(N_CORES):
        wt_i = np.zeros((d, cp), ml_dtypes.bfloat16)
        wt_i[:, :c_per] = wT[:, i * c_per:(i + 1) * c_per].astype(
            ml_dtypes.bfloat16)
        in_maps.append({"emb": emb, "wt": wt_i, "wlab": wlab})
    return in_maps


_CACHED_NC = None


def kernel(embeddings, weight, labels):
    global _CACHED_NC
    if _CACHED_NC is None:
        _CACHED_NC = build_graph()
    in_maps = make_in_maps(embeddings, weight, labels)
    res = run_bass_kernel_spmd(_CACHED_NC, in_maps,
                               core_ids=list(range(N_CORES)), trace=False)
    val = np.asarray(res.results[0]["out"], np.float32).reshape(())
    return val


if __name__ == "__main__":
    rng = np.random.default_rng(0)
    e = rng.standard_normal((B, D)).astype(np.float32)
    w = (rng.random((C, D), np.float32) - 0.5) * 0.015
    l = rng.integers(0, C, B).astype(np.int64)
    print(kernel(e, w, l))



# revision 6
# speedup vs baseline: 2.6256x; 2.6256x over previous
"""ArcFace loss on 8 TRN2 NeuronCores (vocab/tensor-parallel over classes).

Math (per reference):
    cos = normalize(emb) @ normalize(W).T            [B, C]
    phi applied at the label column only (ArcFace margin)
    loss = mean CE(64 * modified cos, labels)

Device-side work is reduced to the two irreducible O(B*C) pieces: the
big cosine matmul and the per-row sum of exp(64*cos - 16).  Everything
else is O(B*D) or O(C*D) staging done on the host:

  host stage:  normalize rows of emb and W, scale by 16, cast to
               fp8e4m3, lay out transposed (contraction dim on
               partitions) for the PE; per-core class shard padded to
               12800 = 25 blocks of 512.
  device:      for each class-block: 2 DoubleRow fp8 matmuls
               (K=256 each) accumulating into PSUM, then one
               Activation Exp over a 4-bank super-block with
               accum_out producing per-row partial sums.  The only
               output is a [128, 8] tile of per-row partial sum-exps
               (with the constant -16 flash bias folded in).
  host final:  sum partials across the 8 cores, apply the exact fp32
               label-column correction (replace exp(64*cos_l) by
               exp(64*phi_l)), subtract the zero-pad contribution,
               take log and the batch mean.

The fp8 (e4m3, TRN max 240) quantization of the two normalized
operands gives ~1.7e-3 cosine noise -> ~1e-4 relative loss error,
far inside the 2e-2 gate, and doubles PE throughput via DoubleRow.
"""

import math
import numpy as np
import ml_dtypes

import concourse.mybir as mybir
from concourse import bacc, tile
from concourse.bass_utils import run_bass_kernel_spmd

N_CORES = 8
B = 1024
D = 512
C = 100000
C_PER = C // N_CORES          # 12500
CP = 12800                    # per-core classes padded to 25 * 512
CB = 512                      # matmul free-dim block (one PSUM bank)
SUPER_CB = 4                  # class blocks per exp super-block (4 banks)
SCALE = 64.0
MARGIN = 0.5
EXP_BIAS = -16.0
SE = 16.0                     # fp8 pre-scale for normalized embeddings
SW = 16.0                     # fp8 pre-scale for normalized weights

M_TILES = B // 128            # 8
K_CHUNKS = D // 128           # 4
K_PAIRS = K_CHUNKS // 2       # 2 DoubleRow K=256 chunks
N_BLOCKS = CP // CB           # 25

FP32 = mybir.dt.float32
BF16 = mybir.dt.bfloat16
FP8 = mybir.dt.float8e4
AF = mybir.ActivationFunctionType
ALU = mybir.AluOpType
X = mybir.AxisListType.X
PERF = mybir.MatmulPerfMode.DoubleRow

COS_M = math.cos(MARGIN)
SIN_M = math.sin(MARGIN)
TH = math.cos(math.pi - MARGIN)
MM = math.sin(math.pi - MARGIN) * MARGIN


def _supers(n_blocks: int, super_cb: int):
    """[(first_block, n_cb), ...] covering n_blocks class blocks."""
    out = []
    b = 0
    while b < n_blocks:
        n = min(super_cb, n_blocks - b)
        out.append((b, n))
        b += n
    return out


def build_graph(b=B, cp=CP, super_cb=SUPER_CB):
    m_tiles = b // 128
    n_blocks = cp // CB
    supers = _supers(n_blocks, super_cb)
    n_sup = len(supers)

    nc = bacc.Bacc("TRN2", target_bir_lowering=False, debug=False,
                   num_devices=N_CORES)
    # ehT: normalized*SE embeddings, transposed: [p, k, b] = ehat[b, k*128+p]
    ehT_d = nc.dram_tensor("ehT", [128, K_CHUNKS * b], FP8,
                           kind="ExternalInput")
    # wt: per-core shard, block-major: [(cb p), (k x)] = what[cb*512+x, k*128+p]
    wt_d = nc.dram_tensor("wt", [n_blocks * 128, K_CHUNKS * CB], FP8,
                          kind="ExternalInput")
    out_d = nc.dram_tensor("out", [128, m_tiles], FP32, kind="ExternalOutput")

    ehT_ap = ehT_d.ap()
    wt_ap = wt_d.ap()

    with tile.TileContext(nc) as tc:
        with (
            tc.tile_pool(name="persist", bufs=1) as pp,
            tc.tile_pool(name="wpool", bufs=3) as wp,
            tc.tile_pool(name="expool", bufs=3) as exp_p,
            tc.tile_pool(name="small", bufs=2) as sp,
            tc.tile_pool(name="psum", bufs=2, space="PSUM") as ps,
        ):
            bias_n = pp.tile([128, 1], FP32, tag="bias_n")
            nc.vector.memset(bias_n[:], EXP_BIAS)
            ehT = pp.tile([128, K_CHUNKS, b], FP8, tag="ehT")
            nc.scalar.dma_start(
                ehT[:], ehT_ap[:, :].rearrange("p (k x) -> p k x",
                                               k=K_CHUNKS))
            acc = [pp.tile([128, n_sup], FP32, tag=f"acc{m}", name=f"acc{m}")
                   for m in range(m_tiles)]

            wh = {}

            def fetch(si):
                cb0, ncb = supers[si]
                wt_t = wp.tile([128, super_cb, K_CHUNKS, CB], FP8, tag="wt",
                               name=f"wt{si}")
                nc.sync.dma_start(
                    wt_t[:, :ncb, :, :],
                    wt_ap[cb0 * 128:(cb0 + ncb) * 128, :].rearrange(
                        "(c p) (k x) -> p c k x", p=128, k=K_CHUNKS))
                wh[si] = wt_t

            fetch(0)
            fetch(1)

            for si, (cb0, ncb) in enumerate(supers):
                if si + 2 < n_sup:
                    fetch(si + 2)
                wt_t = wh[si]
                for m in range(m_tiles):
                    pg = ps.tile([128, super_cb * CB], FP32, tag="pg",
                                 name=f"pg{si}_{m}")
                    for cb in range(ncb):
                        for j in range(K_PAIRS):
                            nc.tensor.matmul(
                                pg[:, cb * CB:(cb + 1) * CB],
                                ehT[:, 2 * j:2 * j + 2,
                                    m * 128:(m + 1) * 128],
                                wt_t[:, cb, 2 * j:2 * j + 2, :],
                                start=(j == 0), stop=(j == K_PAIRS - 1),
                                perf_mode=PERF)
                    ex = exp_p.tile([128, super_cb * CB], BF16, tag="ex",
                                    name=f"ex{si}_{m}")
                    nc.scalar.activation(
                        ex[:, :ncb * CB], pg[:, :ncb * CB], AF.Exp,
                        bias=bias_n[:], scale=SCALE / (SE * SW),
                        accum_out=acc[m][:, si:si + 1])

            sred = sp.tile([128, m_tiles], FP32, tag="sred")
            for m in range(m_tiles):
                nc.vector.tensor_reduce(sred[:, m:m + 1], acc[m][:],
                                        X, ALU.add)
            nc.sync.dma_start(out_d.ap()[:, :], sred[:])

    nc.compile()
    return nc


def make_in_maps(embeddings, weight, labels, b=B, cp=CP):
    """Host staging: normalize, fp8-cast, transpose, shard; plus the fp64
    label-correction context used by finalize()."""
    emb = np.asarray(embeddings, np.float64)
    w = np.asarray(weight, np.float64)
    lab = np.asarray(labels).astype(np.int64)
    c, d = w.shape

    ehat = emb / np.maximum(np.linalg.norm(emb, axis=1, keepdims=True), 1e-12)
    what = w / np.maximum(np.linalg.norm(w, axis=1, keepdims=True), 1e-12)

    e8 = (ehat * SE).astype(np.float32).astype(ml_dtypes.float8_e4m3)
    # [B, D] -> [B, K, 128] -> [128, K, B]
    ehT8 = np.ascontiguousarray(
        e8.reshape(b, K_CHUNKS, 128).transpose(2, 1, 0)).reshape(128, -1)

    w8 = (what * SW).astype(np.float32).astype(ml_dtypes.float8_e4m3)
    c_per = c // N_CORES
    in_maps = []
    for i in range(N_CORES):
        shard = np.zeros((cp, d), ml_dtypes.float8_e4m3)
        shard[:c_per] = w8[i * c_per:(i + 1) * c_per]
        # [CP, D] -> [NB, 512, K, 128] -> [NB, 128, K, 512] -> 2D
        wt8 = np.ascontiguousarray(
            shard.reshape(N_BLOCKS, CB, K_CHUNKS, 128).transpose(0, 3, 2, 1)
        ).reshape(N_BLOCKS * 128, K_CHUNKS * CB)
        in_maps.append({"ehT": ehT8, "wt": wt8})

    # fp64 label-column correction (exact cos at the label position)
    cos_l = np.einsum('bd,bd->b', ehat, what[lab])
    sin_l = np.sqrt(np.clip(1.0 - cos_l * cos_l, 0.0, 1.0))
    phi = cos_l * COS_M - sin_l * SIN_M
    phi = np.where(cos_l > TH, phi, cos_l - MM)
    t = SCALE * phi
    delta = np.exp(t + EXP_BIAS) - np.exp(SCALE * cos_l + EXP_BIAS)
    n_pad_total = (cp - c_per) * N_CORES
    host_ctx = {"t": t, "delta": delta,
                "pad": n_pad_total * math.exp(EXP_BIAS)}
    return in_maps, host_ctx


def finalize(core_outs, host_ctx, b=B):
    """core_outs: list of [128, M_TILES] per-core partial sum-exp tiles."""
    total = np.zeros((128, M_TILES), np.float64)
    for o in core_outs:
        total += np.asarray(o, np.float64)
    # row b = m*128 + p  ->  flatten [p, m] with order p-major per column
    sum_dev = total.transpose(1, 0).reshape(b)
    sum_all = sum_dev + host_ctx["delta"] - host_ctx["pad"]
    loss = np.mean(np.log(sum_all) - EXP_BIAS - host_ctx["t"])
    return np.float32(loss)


_CACHED_NC = None


def kernel(embeddings, weight, labels):
    global _CACHED_NC
    if _CACHED_NC is None:
        _CACHED_NC = build_graph()
    in_maps, host_ctx = make_in_maps(embeddings, weight, labels)
    res = run_bass_kernel_spmd(_CACHED_NC, in_maps,
                               core_ids=list(range(N_CORES)), trace=False)
    return finalize([r["out"] for r in res.results], host_ctx)


if __name__ == "__main__":
    rng = np.random.default_rng(0)
    e = rng.standard_normal((B, D)).astype(np.float32)
    w = (rng.random((C, D)).astype(np.float32) - 0.5) * 0.015
    l = rng.integers(0, C, B).astype(np.int64)
    print(kernel(e, w, l))
